# revision 6
# baseline (speedup 1.0000x reference)
"""Time-varying FIR (AllZeroDigitalFilter) on 8 TRN2 NeuronCores.

Hybrid 3-engine design:

Path 1 (PE / Tensor engine), frames: seq0[0:125) + all of seq1 (1125/core):
  Per frame g one self-loading matmul: stationary lhsT[p,i] = x[80g+i-(49-p)]
  (a [50 taps x 80 positions] Toeplitz slice of a shifted-copy SBUF
  buffer built by strided DMAs with partition stride +1; tap order is
  reversed so ht rows are flipped to match), moving rhs = hT[:, g:g+2]
  (filters h_g, h_{g+1}) -> PSUM [80, 2] fp32: A_g[i], B_g[i]. Per
  125-frame chunk the interpolation blend y = w0[i]*A + w1[i]*B runs
  as 2 wide DVE ops (per-partition ramp scalars, stride-2 PSUM APs),
  then a PE-transpose [80,125]->[125,80] puts y in frame-major order,
  ACT evacuates PSUM->SBUF fp32, and one contiguous DMA stores the
  chunk. ldweights dominates PE time (~67ns/frame: stationary load
  cost scales with columns). The 50x shifted-copy replication is ~9MB
  of DMA traffic, so each chunk's load is split across two DGE queues
  (SP half + GpSimd half) to spread DMA-engine load; 6 chunk buffers
  deep so the PE never starves.

Path 2 (DVE+ACT machinery), frames seq0[125:1000):
  fp16 "C-decomposition": C_k[i'] = sum_j h[k,j] x[(k-1)P+i'-j],
  i' in [0,160); y[kP+i] = w0[i]*C_k[80+i] + w1[i]*C_{k+1}[i].
  Per 126-row tile, N_DVE taps run as scalar_tensor_tensor chains on
  Vector; the other taps are Scalar-engine products folded by a fp16
  tensor-tensor halving tree on Vector. Cross-partition combine via
  partition-shifted SBUF->SBUF DMA + one add emitting fp32.

The paths share engines: DVE runs one PE blend after each path-2
tile; ACT runs one PSUM evacuation per tile; SP and GpSimd split the
DMA issue load so neither path's waits block the other's transfers.

Sync design note: cumulative thresholds on a shared DMA semaphore are
unsound with >1 DMA in flight (per-SDMA-engine completion skew lets a
later tile's increments satisfy an earlier tile's threshold). Buffer-
parity semaphores make every threshold equal to the maximum possible
increment count at wait time, so a fired wait implies full completion.
"""

import sys

for p in ("/opt/trn_rl_repo", "/root/.axon_site/_ro/trn_rl_repo"):
    if p not in sys.path:
        sys.path.append(p)

import numpy as np
import concourse.bass as bass
import concourse.mybir as mybir
from concourse.ap import AP
from concourse.bass_utils import run_bass_kernel_spmd

B, T = 16, 80000
P, D = 80, 50  # frame period, taps
N = T // P  # 1000 frames
W2 = 2 * P + D - 1  # 209: extended window for the 160-wide C rows
NCORES = 8
S = B // NCORES  # sequences per core
FO = 125  # output frames per tile (path 2) / per PE chunk
FT = FO + 1  # C-rows per tile (tiles overlap by 1 row)
PAD = D - 1 + P  # front pad so windows are in-bounds: 129
TPC = N * P + W2 + 2  # padded x length (+2 slack for the odd-offset copy)

F16 = mybir.dt.float16
FP32 = mybir.dt.float32

N_DVE = 26  # path-2 taps computed on the Vector engine

# --- PE path layout ---
PE_CHUNKS = [(0, 0)] + [(1, g0) for g0 in range(0, N, FO)]  # (seq, first frame)
NCH = len(PE_CHUNKS)  # 9 chunks x 125 frames
PE_S0_FRAMES = 125  # seq0 frames handled by the PE path
NT_BASE = (N - PE_S0_FRAMES) // FO  # 7 path-2 tiles, all seq0
HTS = 1008  # ht column stride per sequence
WXS = FO * P  # shifted-x chunk buffer width: 10000
NXS = 6  # chunk buffers (DMA runway depth)
HAF = D // 2  # rows per DMA half-split

_nc_cache = {}


def build_nc():
    if "nc" in _nc_cache:
        return _nc_cache["nc"]
    nc = bass.Bass()
    xp_ext = nc.declare_dram_parameter("xp", [S, TPC], F16, isOutput=False)
    hc_ext = nc.declare_dram_parameter("hc", [S, N + 1, D], FP32, isOutput=False)
    rr_ext = nc.declare_dram_parameter("rr", [128, 2 * P], F16, isOutput=False)
    ht_ext = nc.declare_dram_parameter("ht", [D, S * HTS], F16, isOutput=False)
    id_ext = nc.declare_dram_parameter("idt", [128, 128], F16, isOutput=False)
    wv_ext = nc.declare_dram_parameter("wv", [128, 2], FP32, isOutput=False)
    out_ext = nc.declare_dram_parameter("out", [S, T], FP32, isOutput=True)

    from contextlib import ExitStack

    with ExitStack() as _ctx:
        ec = _ctx.enter_context
        # --- path 2 (DVE+ACT) buffers ---
        xa0 = ec(nc.sbuf_tensor([FT, W2], F16))
        xa1 = ec(nc.sbuf_tensor([FT, W2], F16))
        xb0 = ec(nc.sbuf_tensor([FT, W2], F16))
        xb1 = ec(nc.sbuf_tensor([FT, W2], F16))
        hh0 = ec(nc.sbuf_tensor([FT, D], FP32))
        hh1 = ec(nc.sbuf_tensor([FT, D], FP32))
        acc0 = ec(nc.sbuf_tensor([FT, 2 * P], F16))
        acc1 = ec(nc.sbuf_tensor([FT, 2 * P], F16))
        vt = ec(nc.sbuf_tensor([FT, 2 * P], F16))
        vs = ec(nc.sbuf_tensor([FO, P], F16))
        y0 = ec(nc.sbuf_tensor([FO, P], FP32))
        y1 = ec(nc.sbuf_tensor([FO, P], FP32))
        rrt = ec(nc.sbuf_tensor([128, 2 * P], F16))
        ramp_sem = ec(nc.semaphore("ramp_sem"))
        dma_e = ec(nc.semaphore("dma_e"))
        dma_o = ec(nc.semaphore("dma_o"))
        v_sem = ec(nc.semaphore("v_sem"))
        vs_sem = ec(nc.semaphore("vs_sem"))
        ya_sem = ec(nc.semaphore("ya_sem"))
        out_e = ec(nc.semaphore("out_e"))
        out_o = ec(nc.semaphore("out_o"))
        act_sem = ec(nc.semaphore("act_sem"))
        N_ACT = D - N_DVE
        NSLOT = 32  # padded to a power of two for the in-place halving tree
        assert N_ACT <= NSLOT
        prb0 = ec(nc.sbuf_tensor([FT, NSLOT * 2 * P], F16))
        prb1 = ec(nc.sbuf_tensor([FT, NSLOT * 2 * P], F16))
        prb = [prb0, prb1]

        # --- PE path buffers ---
        xs = [ec(nc.sbuf_tensor(f"xs{i}", [D, WXS], F16)) for i in range(NXS)]
        htt = ec(nc.sbuf_tensor([D, S * HTS], F16))
        idt = ec(nc.sbuf_tensor([128, 128], F16))
        wvt = ec(nc.sbuf_tensor([128, 2], FP32))
        t1b = ec(nc.sbuf_tensor([P, 128], F16))
        yph = [ec(nc.sbuf_tensor(f"yph{i}", [P, 128], F16)) for i in range(2)]
        yo = [ec(nc.sbuf_tensor(f"yo{i}", [FO, P], FP32)) for i in range(2)]
        pab = [ec(nc.psum_tensor(f"pab{i}", [P, 2 * FO], FP32)) for i in range(4)]
        pT = [ec(nc.psum_tensor(f"pT{i}", [FO, P], F16)) for i in range(2)]
        hts = ec(nc.semaphore("hts"))
        wvs = ec(nc.semaphore("wvs"))
        ids = ec(nc.semaphore("ids"))
        xsd = [ec(nc.semaphore(f"xsd{i}")) for i in range(NXS)]
        pe_mm = ec(nc.semaphore("pe_mm"))
        pe_tr = ec(nc.semaphore("pe_tr"))
        bl_sem = ec(nc.semaphore("bl_sem"))
        ev_sem = ec(nc.semaphore("ev_sem"))
        yst = [ec(nc.semaphore(f"yst{i}")) for i in range(2)]

        block = ec(nc.Block())
        xa = [xa0, xa1]
        xb = [xb0, xb1]
        hh = [hh0, hh1]
        yt = [y0, y1]
        dma_s = [dma_e, dma_o]
        out_s = [out_e, out_o]

        def ci_of(t):
            return t + 1  # path-2 tile t covers seq0 frames [(t+1)*FO, (t+2)*FO)

        def ydst(t):
            ci = ci_of(t)
            return AP(
                tensor=out_ext[:].tensor,
                offset=0 * T + ci * FO * P,
                ap=[[P, FO], [1, P]],
            )

        def xs_half(eng, c, r0, r1):
            # rows r0:r1 of chunk c's shifted-x buffer; partition p holds
            # x shifted by tap j = D-1-p (ht rows are flipped to match)
            s, g0 = PE_CHUNKS[c]
            src = AP(
                tensor=xp_ext[:].tensor,
                offset=s * TPC + PAD + g0 * P - (D - 1) + r0,
                ap=[[1, r1 - r0], [1, WXS]],
            )
            eng.dma_start(xs[c % NXS][r0:r1, 0:WXS], src).then_inc(xsd[c % NXS], 16)

        @block.sync
        def _(sync):
            sync.dma_start(htt[:], ht_ext[:]).then_inc(hts, 16)
            xs_half(sync, 0, 0, HAF)
            sync.dma_start(wvt[:], wv_ext[:]).then_inc(wvs, 16)
            sync.dma_start(idt[:], id_ext[:]).then_inc(ids, 16)
            for c in range(1, NXS):
                xs_half(sync, c, 0, HAF)

            def y_store(c):
                s, g0 = PE_CHUNKS[c]
                dst = AP(
                    tensor=out_ext[:].tensor,
                    offset=s * T + g0 * P,
                    ap=[[P, FO], [1, P]],
                )
                sync.dma_start(dst, yo[c % 2][0:FO, 0:P]).then_inc(yst[c % 2], 16)

            for t in range(NT_BASE):
                ci = ci_of(t)
                b = t % 2
                k0 = ci * FO
                if t >= 2:
                    sync.wait_ge(v_sem, t - 1)  # WAR: tile t-2 read its inputs
                src_a = AP(
                    tensor=xp_ext[:].tensor,
                    offset=0 * TPC + k0 * P,
                    ap=[[P, FT], [1, W2]],
                )
                src_b = AP(
                    tensor=xp_ext[:].tensor,
                    offset=0 * TPC + k0 * P + 1,
                    ap=[[P, FT], [1, W2]],
                )
                sync.dma_start(xa[b][:], src_a).then_inc(dma_s[b], 16)
                sync.dma_start(xb[b][:], src_b).then_inc(dma_s[b], 16)
                sync.dma_start(hh[b][:], hc_ext[0, k0 : k0 + FT, :]).then_inc(
                    dma_s[b], 16
                )
                if t == 0:
                    sync.dma_start(rrt[:], rr_ext[:]).then_inc(ramp_sem, 16)
                if t >= 1:
                    # partition-shift copy of V rows 1..FT for tile t-1
                    sync.wait_ge(v_sem, t)
                    sync.dma_start(vs[:], vt[1:FT, 0:P]).then_inc(vs_sem, 16)
                if t >= 2:
                    # store y of tile t-2
                    sync.wait_ge(ya_sem, t - 1)
                    sync.dma_start(ydst(t - 2), yt[(t - 2) % 2][:]).then_inc(
                        out_s[(t - 2) % 2], 16
                    )
                # --- PE path interleaves ---
                if 1 <= t <= NCH - NXS:
                    c = t + NXS - 1  # Xs chunks 6..8 at tiles 1..3
                    sync.wait_ge(pe_mm, c - (NXS - 1))  # buffer c%NXS free
                    xs_half(sync, c, 0, HAF)
                if t >= 2:
                    c = t - 2  # stores for chunks 0..4
                    sync.wait_ge(ev_sem, c + 1)
                    y_store(c)

            # tail: last tile's shift + remaining stores
            tl = NT_BASE - 1
            sync.wait_ge(v_sem, NT_BASE)
            sync.dma_start(vs[:], vt[1:FT, 0:P]).then_inc(vs_sem, 16)
            sync.wait_ge(ya_sem, NT_BASE - 1)
            sync.dma_start(ydst(tl - 1), yt[(tl - 1) % 2][:]).then_inc(
                out_s[(tl - 1) % 2], 16
            )
            sync.wait_ge(ya_sem, NT_BASE)
            sync.dma_start(ydst(tl), yt[tl % 2][:]).then_inc(out_s[tl % 2], 16)
            for c in range(NT_BASE - 2, NCH):
                sync.wait_ge(ev_sem, c + 1)
                y_store(c)
            sync.wait_ge(out_s[tl % 2], 16 * (tl // 2 + 1))
            sync.wait_ge(out_s[1 - tl % 2], 16 * ((tl - 1) // 2 + 1))
            sync.wait_ge(yst[0], 16 * ((NCH + 1) // 2))
            sync.wait_ge(yst[1], 16 * (NCH // 2))

        @block.gpsimd
        def _(gp):
            # second DGE queue for the other half of each chunk's rows
            for c in range(NCH):
                if c >= NXS:
                    gp.wait_ge(pe_mm, c - (NXS - 1))
                xs_half(gp, c, HAF, D)

        def src_for(buf_pair, b, j):
            # slice of the extended window for tap j, 4B-aligned via the
            # one-element-shifted copy when the natural offset is odd
            off = D - 1 - j
            if off % 2 == 0:
                return buf_pair[0][b][:, off : off + 2 * P]
            return buf_pair[1][b][:, off - 1 : off - 1 + 2 * P]

        @block.vector
        def _(vector):
            def conv(t):
                b = t % 2
                accs = [acc0, acc1]
                vector.wait_ge(dma_s[b], 48 * (t // 2 + 1))
                vector.tensor_scalar_mul(acc0[:], src_for((xa, xb), b, 0), hh[b][:, 0:1])
                cur = 0
                for j in range(1, N_DVE):
                    nxt = 1 - cur
                    vector.scalar_tensor_tensor(
                        out=accs[nxt][:],
                        in0=src_for((xa, xb), b, j),
                        scalar=hh[b][:, j : j + 1],
                        in1=accs[cur][:],
                        op0=mybir.AluOpType.mult,
                        op1=mybir.AluOpType.add,
                    )
                    cur = nxt
                # fold in the ACT-engine products
                vector.wait_ge(act_sem, t + 1)
                if N_ACT > 16:
                    extra = N_ACT - 16
                    vector.tensor_tensor(
                        out=prb[b][:, 0 : extra * 2 * P],
                        in0=prb[b][:, 0 : extra * 2 * P],
                        in1=prb[b][:, 16 * 2 * P : N_ACT * 2 * P],
                        op=mybir.AluOpType.add,
                    )
                    width = 16 * 2 * P
                else:
                    width = NSLOT * 2 * P
                while width > 2 * P:
                    half = width // 2
                    vector.tensor_tensor(
                        out=prb[b][:, 0:half],
                        in0=prb[b][:, 0:half],
                        in1=prb[b][:, half:width],
                        op=mybir.AluOpType.add,
                    )
                    width = half
                nxt = 1 - cur
                vector.tensor_tensor(
                    out=accs[nxt][:],
                    in0=accs[cur][:],
                    in1=prb[b][:, 0 : 2 * P],
                    op=mybir.AluOpType.add,
                )
                cur = nxt
                return accs[cur]

            def blend(c):
                buf = c % 4
                vector.wait_ge(pe_mm, c + 1)
                if c >= 2:
                    vector.wait_ge(pe_tr, c - 1)  # yph[c%2] WAR
                if c == 0:
                    vector.wait_ge(wvs, 16)
                vector.tensor_scalar_mul(
                    t1b[0:P, 0:FO], pab[buf][0:P, 1 : 2 * FO : 2], wvt[0:P, 1:2]
                )
                vector.scalar_tensor_tensor(
                    out=yph[c % 2][0:P, 0:FO],
                    in0=pab[buf][0:P, 0 : 2 * FO : 2],
                    scalar=wvt[0:P, 0:1],
                    in1=t1b[0:P, 0:FO],
                    op0=mybir.AluOpType.mult,
                    op1=mybir.AluOpType.add,
                ).then_inc(bl_sem, 1)

            if N_ACT <= 16:
                for pp in range(2):
                    vector.memset(prb[pp][:, N_ACT * 2 * P : NSLOT * 2 * P], 0.0)
            for t in range(NT_BASE):
                fin = conv(t)
                if t == 0:
                    vector.wait_ge(ramp_sem, 16)
                if t >= 1:
                    # combine tile t-1: y = V[0:FO, 80:160] + Vs
                    vector.wait_ge(vs_sem, 16 * t)
                    if t - 1 >= 2:
                        vector.wait_ge(out_s[(t - 1) % 2], 16 * ((t - 1) // 2))
                    vector.tensor_tensor(
                        out=yt[(t - 1) % 2][:],
                        in0=vt[0:FO, P : 2 * P],
                        in1=vs[:],
                        op=mybir.AluOpType.add,
                    ).then_inc(ya_sem, 1)
                # V_t = C_t * rr
                vector.tensor_tensor(
                    out=vt[:], in0=fin[:], in1=rrt[0:FT, :], op=mybir.AluOpType.mult
                ).then_inc(v_sem, 1)
                # --- PE-path blend interleave: chunk t after tile t ---
                if t < NCH:
                    blend(t)
            # tail combine for last tile
            tl = NT_BASE - 1
            vector.wait_ge(vs_sem, 16 * NT_BASE)
            vector.wait_ge(out_s[tl % 2], 16 * (tl // 2))
            vector.tensor_tensor(
                out=yt[tl % 2][:],
                in0=vt[0:FO, P : 2 * P],
                in1=vs[:],
                op=mybir.AluOpType.add,
            ).then_inc(ya_sem, 1)
            for c in range(NT_BASE, NCH):
                blend(c)

        @block.scalar
        def _(scalar):
            def evac(c):
                scalar.wait_ge(pe_tr, c + 1)
                if c >= 2:
                    scalar.wait_ge(yst[c % 2], 16 * ((c - 2) // 2 + 1))  # yo WAR
                scalar.activation(
                    yo[c % 2][0:FO, 0:P],
                    pT[c % 2][0:FO, 0:P],
                    mybir.ActivationFunctionType.Copy,
                ).then_inc(ev_sem, 1)

            for t in range(NT_BASE):
                b = t % 2
                scalar.wait_ge(dma_s[b], 48 * (t // 2 + 1))
                if t >= 2:
                    scalar.wait_ge(v_sem, t - 1)  # WAR on prb[b] scratch
                for idx, j in enumerate(range(N_DVE, D)):
                    inst = scalar.activation(
                        prb[b][:, idx * 2 * P : (idx + 1) * 2 * P],
                        src_for((xa, xb), b, j),
                        mybir.ActivationFunctionType.Copy,
                        scale=hh[b][:, j : j + 1],
                    )
                    if idx == N_ACT - 1:
                        inst.then_inc(act_sem, 1)
                # --- PE-path evacuation interleave ---
                c = t - 1
                if 0 <= c < NCH:
                    evac(c)
            for c in range(NT_BASE - 1, NCH):
                evac(c)

        @block.tensor
        def _(tensor):
            def do_transpose(c):
                tensor.wait_ge(bl_sem, c + 1)  # yph ready
                if c == 0:
                    tensor.wait_ge(ids, 16)
                if c >= 2:
                    tensor.wait_ge(ev_sem, c - 1)  # pT[c%2] WAR
                tensor.transpose(
                    pT[c % 2][0:FO, 0:P],
                    yph[c % 2][0:P, 0:FO],
                    idt[0:P, 0:P],
                ).then_inc(pe_tr, 1)

            tensor.wait_ge(hts, 16)
            for c in range(NCH):
                s, g0 = PE_CHUNKS[c]
                buf = c % NXS
                if c >= 4:
                    tensor.wait_ge(bl_sem, c - 3)  # pab[c%4] WAR
                tensor.wait_ge(xsd[buf], 32 * (c // NXS + 1))
                for g in range(FO):
                    mm = tensor.matmul(
                        pab[c % 4][0:P, 2 * g : 2 * g + 2],
                        xs[buf][0:D, P * g : P * g + P],
                        htt[0:D, s * HTS + g0 + g : s * HTS + g0 + g + 2],
                        start=True,
                        stop=True,
                    )
                    if g == FO - 1:
                        mm.then_inc(pe_mm, 1)
                if c >= 1:
                    do_transpose(c - 1)
            do_transpose(NCH - 1)

    _nc_cache["nc"] = nc
    return nc


def _prep_core_inputs(x, h):
    x = np.ascontiguousarray(x, dtype=np.float32)
    h = np.ascontiguousarray(h, dtype=np.float32)
    xp = np.zeros((B, TPC), np.float16)
    xp[:, PAD : PAD + T] = x.astype(np.float16)
    hpad = np.ascontiguousarray(np.concatenate([h, h[:, -1:, :]], axis=1))  # (B,N+1,D) f32
    w1 = (np.arange(P, dtype=np.float32) / P).astype(np.float16)
    w0 = (1.0 - np.arange(P, dtype=np.float32) / P).astype(np.float16)
    rr = np.broadcast_to(np.concatenate([w1, w0])[None, :], (128, 2 * P))
    rr = np.ascontiguousarray(rr)
    hpad16 = hpad.astype(np.float16)  # (B, N+1, D)
    idt = np.eye(128, dtype=np.float16)
    wv = np.zeros((128, 2), np.float32)
    wv[0:P, 0] = 1.0 - np.arange(P, dtype=np.float32) / P
    wv[0:P, 1] = np.arange(P, dtype=np.float32) / P
    in_maps = []
    for c in range(NCORES):
        sl = slice(c * S, (c + 1) * S)
        ht = np.zeros((D, S * HTS), np.float16)
        for s in range(S):
            # row p = tap D-1-p, matching the stride +1 shifted-x layout
            ht[:, s * HTS : s * HTS + N + 1] = hpad16[c * S + s].T[::-1, :]
        in_maps.append(
            {
                "xp": xp[sl],
                "hc": hpad[sl],
                "rr": rr,
                "ht": ht,
                "idt": idt,
                "wv": wv,
            }
        )
    return in_maps


def kernel(x, h, **kw):
    nc = build_nc()
    in_maps = _prep_core_inputs(x, h)
    res = run_bass_kernel_spmd(nc, in_maps, core_ids=list(range(NCORES)), **kw)
    out = np.concatenate([res.results[c]["out"] for c in range(NCORES)], axis=0)
    return np.ascontiguousarray(out, dtype=np.float32)


def kernel_traced(x, h, **kw):
    nc = build_nc()
    in_maps = _prep_core_inputs(x, h)
    res = run_bass_kernel_spmd(
        nc, in_maps, core_ids=list(range(NCORES)), trace=True, **kw
    )
    out = np.concatenate([res.results[c]["out"] for c in range(NCORES)], axis=0)
    return np.ascontiguousarray(out, dtype=np.float32), res


# revision 15
# speedup vs baseline: 1.1443x; 1.1443x over previous
"""Time-varying FIR (AllZeroDigitalFilter) on 8 TRN2 NeuronCores.

Hybrid 3-engine design:

Path 1 (PE / Tensor engine), frames: seq0[0:125) + all of seq1 (1125/core):
  Per frame g one self-loading matmul: stationary lhsT[p,i] = x[80g+i-(49-p)]
  (a [50 taps x 80 positions] Toeplitz slice of a shifted-copy SBUF
  buffer built by strided DMAs with partition stride +1; tap order is
  reversed so ht rows are flipped to match), moving rhs = hT[:, g:g+2]
  (filters h_g, h_{g+1}) -> PSUM [80, 2] fp32: A_g[i], B_g[i]. Per
  125-frame chunk the interpolation blend y = w0[i]*A + w1[i]*B runs
  as 2 wide DVE ops (per-partition ramp scalars, stride-2 PSUM APs),
  then a PE-transpose [80,125]->[125,80] puts y in frame-major order,
  ACT evacuates PSUM->SBUF fp32, and one contiguous DMA stores the
  chunk. ldweights dominates PE time (~67ns/frame: stationary load
  cost scales with columns). The 50x shifted-copy replication is ~9MB
  of DMA traffic, so each chunk's load is split across two DGE queues
  (SP half + GpSimd half) to spread DMA-engine load; 6 chunk buffers
  deep so the PE never starves.

Path 2 (DVE+ACT machinery), frames seq0[125:1000):
  fp16 "C-decomposition": C_k[i'] = sum_j h[k,j] x[(k-1)P+i'-j],
  i' in [0,160); y[kP+i] = w0[i]*C_k[80+i] + w1[i]*C_{k+1}[i].
  Per 126-row tile, N_DVE taps run as scalar_tensor_tensor chains on
  Vector; the other taps are Scalar-engine products folded by a fp16
  tensor-tensor halving tree on Vector. Cross-partition combine via
  partition-shifted SBUF->SBUF DMA + one add emitting fp32.

The paths share engines: DVE runs one PE blend after each path-2
tile; ACT runs one PSUM evacuation per tile; SP and GpSimd split the
DMA issue load so neither path's waits block the other's transfers.

Sync design note: cumulative thresholds on a shared DMA semaphore are
unsound with >1 DMA in flight (per-SDMA-engine completion skew lets a
later tile's increments satisfy an earlier tile's threshold). Buffer-
parity semaphores make every threshold equal to the maximum possible
increment count at wait time, so a fired wait implies full completion.
"""

import sys

for p in ("/opt/trn_rl_repo", "/root/.axon_site/_ro/trn_rl_repo"):
    if p not in sys.path:
        sys.path.append(p)

import numpy as np
import concourse.bass as bass
import concourse.mybir as mybir
from concourse.ap import AP
from concourse.bass_utils import run_bass_kernel_spmd

B, T = 16, 80000
P, D = 80, 50  # frame period, taps
N = T // P  # 1000 frames
W2 = 2 * P + D - 1  # 209: extended window for the 160-wide C rows
NCORES = 8
S = B // NCORES  # sequences per core
FO = 125  # output frames per tile (path 2) / per PE chunk
FT = FO + 1  # C-rows per tile (tiles overlap by 1 row)
PAD = D - 1 + P  # front pad so windows are in-bounds: 129
TPC = N * P + W2 + 2  # padded x length (+2 slack for the odd-offset copy)

F16 = mybir.dt.float16
FP32 = mybir.dt.float32

N_DVE = 26  # path-2 taps computed on the Vector engine

# --- PE path layout ---
PE_CHUNKS = [(0, 0)] + [(1, g0) for g0 in range(0, N, FO)]  # (seq, first frame)
NCH = len(PE_CHUNKS)  # 9 chunks x 125 frames
PE_S0_FRAMES = 125  # seq0 frames handled by the PE path
NT_BASE = (N - PE_S0_FRAMES) // FO  # 7 path-2 tiles, all seq0
HTS = 1008  # ht column stride per sequence
# SDMA engine k serves partitions 8k..8k+7, so a 50-partition DMA only uses
# engines 0-6. Each chunk is split into two partition blocks — taps at
# partitions 0:50 (frames 0..GA-1) and 64:114 (frames GA..124) — engaging 14
# engines. Block B matmuls use stationary base partition 64 (legal tile
# position for a <=64-row stationary); ht rows are duplicated there.
GA = 63  # frames served by block A
WXA = GA * P  # block A width: 5040
WXB = (FO - GA) * P  # block B width: 4960
NXS = 8  # chunk buffers (DMA runway depth)

_nc_cache = {}


def build_nc():
    if "nc" in _nc_cache:
        return _nc_cache["nc"]
    nc = bass.Bass()
    xp_ext = nc.declare_dram_parameter("xp", [S, TPC], F16, isOutput=False)
    hc_ext = nc.declare_dram_parameter("hc", [S, N + 1, D], FP32, isOutput=False)
    rr_ext = nc.declare_dram_parameter("rr", [128, 2 * P], F16, isOutput=False)
    ht_ext = nc.declare_dram_parameter("ht", [64 + D, S * HTS], F16, isOutput=False)
    id_ext = nc.declare_dram_parameter("idt", [128, 128], F16, isOutput=False)
    wv_ext = nc.declare_dram_parameter("wv", [128, 2], FP32, isOutput=False)
    out_ext = nc.declare_dram_parameter("out", [S, T], FP32, isOutput=True)

    from contextlib import ExitStack

    with ExitStack() as _ctx:
        ec = _ctx.enter_context
        # --- path 2 (DVE+ACT) buffers ---
        xa0 = ec(nc.sbuf_tensor([FT, W2], F16))
        xa1 = ec(nc.sbuf_tensor([FT, W2], F16))
        xb0 = ec(nc.sbuf_tensor([FT, W2], F16))
        xb1 = ec(nc.sbuf_tensor([FT, W2], F16))
        hh0 = ec(nc.sbuf_tensor([FT, D], FP32))
        hh1 = ec(nc.sbuf_tensor([FT, D], FP32))
        acc0 = ec(nc.sbuf_tensor([FT, 2 * P], F16))
        acc1 = ec(nc.sbuf_tensor([FT, 2 * P], F16))
        vt = ec(nc.sbuf_tensor([FT, 2 * P], F16))
        vs = ec(nc.sbuf_tensor([FO, P], F16))
        y0 = ec(nc.sbuf_tensor([FO, P], FP32))
        y1 = ec(nc.sbuf_tensor([FO, P], FP32))
        rrt = ec(nc.sbuf_tensor([128, 2 * P], F16))
        ramp_sem = ec(nc.semaphore("ramp_sem"))
        dma_e = ec(nc.semaphore("dma_e"))
        dma_o = ec(nc.semaphore("dma_o"))
        v_sem = ec(nc.semaphore("v_sem"))
        vs_sem = ec(nc.semaphore("vs_sem"))
        ya_sem = ec(nc.semaphore("ya_sem"))
        out_e = ec(nc.semaphore("out_e"))
        out_o = ec(nc.semaphore("out_o"))
        act_sem = ec(nc.semaphore("act_sem"))
        N_ACT = D - N_DVE
        NSLOT = 32  # padded to a power of two for the in-place halving tree
        assert N_ACT <= NSLOT
        prb0 = ec(nc.sbuf_tensor([FT, NSLOT * 2 * P], F16))
        prb1 = ec(nc.sbuf_tensor([FT, NSLOT * 2 * P], F16))
        prb = [prb0, prb1]

        # --- PE path buffers ---
        xs = [ec(nc.sbuf_tensor(f"xs{i}", [64 + D, WXA], F16)) for i in range(NXS)]
        htt = ec(nc.sbuf_tensor([64 + D, S * HTS], F16))
        idt = ec(nc.sbuf_tensor([128, 128], F16))
        wvt = ec(nc.sbuf_tensor([128, 2], FP32))
        t1b = ec(nc.sbuf_tensor([P, 128], F16))
        yph = [ec(nc.sbuf_tensor(f"yph{i}", [P, 128], F16)) for i in range(2)]
        yo = [ec(nc.sbuf_tensor(f"yo{i}", [FO, P], FP32)) for i in range(2)]
        pab = [ec(nc.psum_tensor(f"pab{i}", [P, 2 * FO], FP32)) for i in range(4)]
        pT = [ec(nc.psum_tensor(f"pT{i}", [FO, P], F16)) for i in range(2)]
        hts = ec(nc.semaphore("hts"))
        wvs = ec(nc.semaphore("wvs"))
        ids = ec(nc.semaphore("ids"))
        xsd = [ec(nc.semaphore(f"xsd{i}")) for i in range(NXS)]
        pe_mm = ec(nc.semaphore("pe_mm"))
        pe_tr = ec(nc.semaphore("pe_tr"))
        bl_sem = ec(nc.semaphore("bl_sem"))
        ev_sem = ec(nc.semaphore("ev_sem"))
        yst = [ec(nc.semaphore(f"yst{i}")) for i in range(2)]

        block = ec(nc.Block())
        xa = [xa0, xa1]
        xb = [xb0, xb1]
        hh = [hh0, hh1]
        yt = [y0, y1]
        dma_s = [dma_e, dma_o]
        out_s = [out_e, out_o]

        def ci_of(t):
            return t + 1  # path-2 tile t covers seq0 frames [(t+1)*FO, (t+2)*FO)

        def ydst(t):
            ci = ci_of(t)
            return AP(
                tensor=out_ext[:].tensor,
                offset=0 * T + ci * FO * P,
                ap=[[P, FO], [1, P]],
            )

        def xs_dma(eng, c):
            # two partition blocks per chunk; partition p (resp. 64+p) holds
            # x shifted by tap j = D-1-p (ht rows are flipped to match)
            s, g0 = PE_CHUNKS[c]
            src_a = AP(
                tensor=xp_ext[:].tensor,
                offset=s * TPC + PAD + g0 * P - (D - 1),
                ap=[[1, D], [1, WXA]],
            )
            eng.dma_start(xs[c % NXS][0:D, 0:WXA], src_a).then_inc(xsd[c % NXS], 16)
            src_b = AP(
                tensor=xp_ext[:].tensor,
                offset=s * TPC + PAD + (g0 + GA) * P - (D - 1),
                ap=[[1, D], [1, WXB]],
            )
            eng.dma_start(xs[c % NXS][64 : 64 + D, 0:WXB], src_b).then_inc(
                xsd[c % NXS], 16
            )

        @block.sync
        def _(sync):
            sync.dma_start(htt[:], ht_ext[:]).then_inc(hts, 16)
            xs_dma(sync, 0)
            sync.dma_start(wvt[:], wv_ext[:]).then_inc(wvs, 16)
            sync.dma_start(idt[:], id_ext[:]).then_inc(ids, 16)
            for c in range(1, NXS):
                xs_dma(sync, c)

            def y_store(c):
                s, g0 = PE_CHUNKS[c]
                dst = AP(
                    tensor=out_ext[:].tensor,
                    offset=s * T + g0 * P,
                    ap=[[P, FO], [1, P]],
                )
                sync.dma_start(dst, yo[c % 2][0:FO, 0:P]).then_inc(yst[c % 2], 16)

            for t in range(NT_BASE):
                ci = ci_of(t)
                b = t % 2
                k0 = ci * FO
                if t >= 2:
                    sync.wait_ge(v_sem, t - 1)  # WAR: tile t-2 read its inputs
                src_a = AP(
                    tensor=xp_ext[:].tensor,
                    offset=0 * TPC + k0 * P,
                    ap=[[P, FT], [1, W2]],
                )
                src_b = AP(
                    tensor=xp_ext[:].tensor,
                    offset=0 * TPC + k0 * P + 1,
                    ap=[[P, FT], [1, W2]],
                )
                sync.dma_start(xa[b][:], src_a).then_inc(dma_s[b], 16)
                sync.dma_start(xb[b][:], src_b).then_inc(dma_s[b], 16)
                sync.dma_start(hh[b][:], hc_ext[0, k0 : k0 + FT, :]).then_inc(
                    dma_s[b], 16
                )
                if t == 0:
                    sync.dma_start(rrt[:], rr_ext[:]).then_inc(ramp_sem, 16)
                if t >= 1:
                    # partition-shift copy of V rows 1..FT for tile t-1
                    sync.wait_ge(v_sem, t)
                    sync.dma_start(vs[:], vt[1:FT, 0:P]).then_inc(vs_sem, 16)
                if t >= 2:
                    # store y of tile t-2
                    sync.wait_ge(ya_sem, t - 1)
                    sync.dma_start(ydst(t - 2), yt[(t - 2) % 2][:]).then_inc(
                        out_s[(t - 2) % 2], 16
                    )
                # --- PE path interleaves ---
                if 1 <= t <= NCH - NXS:
                    c = t + NXS - 1  # remaining Xs chunks
                    sync.wait_ge(pe_mm, c - (NXS - 1))  # buffer c%NXS free
                    xs_dma(sync, c)
                if t >= 2:
                    c = t - 2  # stores for chunks 0..4
                    sync.wait_ge(ev_sem, c + 1)
                    y_store(c)

            # tail: last tile's shift + remaining stores
            tl = NT_BASE - 1
            sync.wait_ge(v_sem, NT_BASE)
            sync.dma_start(vs[:], vt[1:FT, 0:P]).then_inc(vs_sem, 16)
            sync.wait_ge(ya_sem, NT_BASE - 1)
            sync.dma_start(ydst(tl - 1), yt[(tl - 1) % 2][:]).then_inc(
                out_s[(tl - 1) % 2], 16
            )
            sync.wait_ge(ya_sem, NT_BASE)
            sync.dma_start(ydst(tl), yt[tl % 2][:]).then_inc(out_s[tl % 2], 16)
            for c in range(NT_BASE - 2, NCH):
                sync.wait_ge(ev_sem, c + 1)
                y_store(c)
            sync.wait_ge(out_s[tl % 2], 16 * (tl // 2 + 1))
            sync.wait_ge(out_s[1 - tl % 2], 16 * ((tl - 1) // 2 + 1))
            sync.wait_ge(yst[0], 16 * ((NCH + 1) // 2))
            sync.wait_ge(yst[1], 16 * (NCH // 2))

        def src_for(buf_pair, b, j):
            # slice of the extended window for tap j, 4B-aligned via the
            # one-element-shifted copy when the natural offset is odd
            off = D - 1 - j
            if off % 2 == 0:
                return buf_pair[0][b][:, off : off + 2 * P]
            return buf_pair[1][b][:, off - 1 : off - 1 + 2 * P]

        @block.vector
        def _(vector):
            def conv(t):
                b = t % 2
                accs = [acc0, acc1]
                vector.wait_ge(dma_s[b], 48 * (t // 2 + 1))
                vector.tensor_scalar_mul(acc0[:], src_for((xa, xb), b, 0), hh[b][:, 0:1])
                cur = 0
                for j in range(1, N_DVE):
                    nxt = 1 - cur
                    vector.scalar_tensor_tensor(
                        out=accs[nxt][:],
                        in0=src_for((xa, xb), b, j),
                        scalar=hh[b][:, j : j + 1],
                        in1=accs[cur][:],
                        op0=mybir.AluOpType.mult,
                        op1=mybir.AluOpType.add,
                    )
                    cur = nxt
                # fold in the ACT-engine products
                vector.wait_ge(act_sem, t + 1)
                if N_ACT > 16:
                    extra = N_ACT - 16
                    vector.tensor_tensor(
                        out=prb[b][:, 0 : extra * 2 * P],
                        in0=prb[b][:, 0 : extra * 2 * P],
                        in1=prb[b][:, 16 * 2 * P : N_ACT * 2 * P],
                        op=mybir.AluOpType.add,
                    )
                    width = 16 * 2 * P
                else:
                    width = NSLOT * 2 * P
                while width > 2 * P:
                    half = width // 2
                    vector.tensor_tensor(
                        out=prb[b][:, 0:half],
                        in0=prb[b][:, 0:half],
                        in1=prb[b][:, half:width],
                        op=mybir.AluOpType.add,
                    )
                    width = half
                nxt = 1 - cur
                vector.tensor_tensor(
                    out=accs[nxt][:],
                    in0=accs[cur][:],
                    in1=prb[b][:, 0 : 2 * P],
                    op=mybir.AluOpType.add,
                )
                cur = nxt
                return accs[cur]

            def blend(c):
                buf = c % 4
                vector.wait_ge(pe_mm, c + 1)
                if c >= 2:
                    vector.wait_ge(pe_tr, c - 1)  # yph[c%2] WAR
                if c == 0:
                    vector.wait_ge(wvs, 16)
                vector.tensor_scalar_mul(
                    t1b[0:P, 0:FO], pab[buf][0:P, 1 : 2 * FO : 2], wvt[0:P, 1:2]
                )
                vector.scalar_tensor_tensor(
                    out=yph[c % 2][0:P, 0:FO],
                    in0=pab[buf][0:P, 0 : 2 * FO : 2],
                    scalar=wvt[0:P, 0:1],
                    in1=t1b[0:P, 0:FO],
                    op0=mybir.AluOpType.mult,
                    op1=mybir.AluOpType.add,
                ).then_inc(bl_sem, 1)

            if N_ACT <= 16:
                for pp in range(2):
                    vector.memset(prb[pp][:, N_ACT * 2 * P : NSLOT * 2 * P], 0.0)
            for t in range(NT_BASE):
                fin = conv(t)
                if t == 0:
                    vector.wait_ge(ramp_sem, 16)
                if t >= 1:
                    # combine tile t-1: y = V[0:FO, 80:160] + Vs
                    vector.wait_ge(vs_sem, 16 * t)
                    if t - 1 >= 2:
                        vector.wait_ge(out_s[(t - 1) % 2], 16 * ((t - 1) // 2))
                    vector.tensor_tensor(
                        out=yt[(t - 1) % 2][:],
                        in0=vt[0:FO, P : 2 * P],
                        in1=vs[:],
                        op=mybir.AluOpType.add,
                    ).then_inc(ya_sem, 1)
                # V_t = C_t * rr
                vector.tensor_tensor(
                    out=vt[:], in0=fin[:], in1=rrt[0:FT, :], op=mybir.AluOpType.mult
                ).then_inc(v_sem, 1)
                # --- PE-path blend interleave: chunk t after tile t ---
                if t < NCH:
                    blend(t)
            # tail combine for last tile
            tl = NT_BASE - 1
            vector.wait_ge(vs_sem, 16 * NT_BASE)
            vector.wait_ge(out_s[tl % 2], 16 * (tl // 2))
            vector.tensor_tensor(
                out=yt[tl % 2][:],
                in0=vt[0:FO, P : 2 * P],
                in1=vs[:],
                op=mybir.AluOpType.add,
            ).then_inc(ya_sem, 1)
            for c in range(NT_BASE, NCH):
                blend(c)

        @block.scalar
        def _(scalar):
            def evac(c):
                scalar.wait_ge(pe_tr, c + 1)
                if c >= 2:
                    scalar.wait_ge(yst[c % 2], 16 * ((c - 2) // 2 + 1))  # yo WAR
                scalar.activation(
                    yo[c % 2][0:FO, 0:P],
                    pT[c % 2][0:FO, 0:P],
                    mybir.ActivationFunctionType.Copy,
                ).then_inc(ev_sem, 1)

            for t in range(NT_BASE):
                b = t % 2
                scalar.wait_ge(dma_s[b], 48 * (t // 2 + 1))
                if t >= 2:
                    scalar.wait_ge(v_sem, t - 1)  # WAR on prb[b] scratch
                for idx, j in enumerate(range(N_DVE, D)):
                    inst = scalar.activation(
                        prb[b][:, idx * 2 * P : (idx + 1) * 2 * P],
                        src_for((xa, xb), b, j),
                        mybir.ActivationFunctionType.Copy,
                        scale=hh[b][:, j : j + 1],
                    )
                    if idx == N_ACT - 1:
                        inst.then_inc(act_sem, 1)
                # --- PE-path evacuation interleave ---
                c = t - 1
                if 0 <= c < NCH:
                    evac(c)
            for c in range(NT_BASE - 1, NCH):
                evac(c)

        @block.tensor
        def _(tensor):
            def do_transpose(c):
                tensor.wait_ge(bl_sem, c + 1)  # yph ready
                if c == 0:
                    tensor.wait_ge(ids, 16)
                if c >= 2:
                    tensor.wait_ge(ev_sem, c - 1)  # pT[c%2] WAR
                tensor.transpose(
                    pT[c % 2][0:FO, 0:P],
                    yph[c % 2][0:P, 0:FO],
                    idt[0:P, 0:P],
                ).then_inc(pe_tr, 1)

            tensor.wait_ge(hts, 16)
            for c in range(NCH):
                s, g0 = PE_CHUNKS[c]
                buf = c % NXS
                if c >= 4:
                    tensor.wait_ge(bl_sem, c - 3)  # pab[c%4] WAR
                tensor.wait_ge(xsd[buf], 32 * (c // NXS + 1))
                for g in range(FO):
                    if g < GA:
                        lhs = xs[buf][0:D, P * g : P * g + P]
                        rhs = htt[0:D, s * HTS + g0 + g : s * HTS + g0 + g + 2]
                    else:
                        lhs = xs[buf][64 : 64 + D, P * (g - GA) : P * (g - GA) + P]
                        rhs = htt[
                            64 : 64 + D, s * HTS + g0 + g : s * HTS + g0 + g + 2
                        ]
                    mm = tensor.matmul(
                        pab[c % 4][0:P, 2 * g : 2 * g + 2],
                        lhs,
                        rhs,
                        start=True,
                        stop=True,
                    )
                    if g == FO - 1:
                        mm.then_inc(pe_mm, 1)
                if c >= 1:
                    do_transpose(c - 1)
            do_transpose(NCH - 1)

    _nc_cache["nc"] = nc
    return nc


def _prep_core_inputs(x, h):
    x = np.ascontiguousarray(x, dtype=np.float32)
    h = np.ascontiguousarray(h, dtype=np.float32)
    xp = np.zeros((B, TPC), np.float16)
    xp[:, PAD : PAD + T] = x.astype(np.float16)
    hpad = np.ascontiguousarray(np.concatenate([h, h[:, -1:, :]], axis=1))  # (B,N+1,D) f32
    w1 = (np.arange(P, dtype=np.float32) / P).astype(np.float16)
    w0 = (1.0 - np.arange(P, dtype=np.float32) / P).astype(np.float16)
    rr = np.broadcast_to(np.concatenate([w1, w0])[None, :], (128, 2 * P))
    rr = np.ascontiguousarray(rr)
    hpad16 = hpad.astype(np.float16)  # (B, N+1, D)
    idt = np.eye(128, dtype=np.float16)
    wv = np.zeros((128, 2), np.float32)
    wv[0:P, 0] = 1.0 - np.arange(P, dtype=np.float32) / P
    wv[0:P, 1] = np.arange(P, dtype=np.float32) / P
    in_maps = []
    for c in range(NCORES):
        sl = slice(c * S, (c + 1) * S)
        ht = np.zeros((64 + D, S * HTS), np.float16)
        for s in range(S):
            # row p = tap D-1-p, matching the stride +1 shifted-x layout;
            # duplicated at partition base 64 for block-B matmuls
            ht[0:D, s * HTS : s * HTS + N + 1] = hpad16[c * S + s].T[::-1, :]
            ht[64 : 64 + D, s * HTS : s * HTS + N + 1] = hpad16[c * S + s].T[::-1, :]
        in_maps.append(
            {
                "xp": xp[sl],
                "hc": hpad[sl],
                "rr": rr,
                "ht": ht,
                "idt": idt,
                "wv": wv,
            }
        )
    return in_maps


def kernel(x, h, **kw):
    nc = build_nc()
    in_maps = _prep_core_inputs(x, h)
    res = run_bass_kernel_spmd(nc, in_maps, core_ids=list(range(NCORES)), **kw)
    out = np.concatenate([res.results[c]["out"] for c in range(NCORES)], axis=0)
    return np.ascontiguousarray(out, dtype=np.float32)


def kernel_traced(x, h, **kw):
    nc = build_nc()
    in_maps = _prep_core_inputs(x, h)
    res = run_bass_kernel_spmd(
        nc, in_maps, core_ids=list(range(NCORES)), trace=True, **kw
    )
    out = np.concatenate([res.results[c]["out"] for c in range(NCORES)], axis=0)
    return np.ascontiguousarray(out, dtype=np.float32), res


# revision 21
# speedup vs baseline: 1.2796x; 1.1183x over previous
"""Time-varying FIR (AllZeroDigitalFilter) on 8 TRN2 NeuronCores.

Hybrid 3-engine design:

Path 1 (PE / Tensor engine), frames: seq0[0:125) + all of seq1 (1125/core):
  Per frame g one self-loading matmul: stationary lhsT[p,i] = x[80g+i-(49-p)]
  (a [50 taps x 80 positions] Toeplitz slice of a shifted-copy SBUF
  buffer built by strided DMAs with partition stride +1; tap order is
  reversed so ht rows are flipped to match), moving rhs = hT[:, g:g+2]
  (filters h_g, h_{g+1}) -> PSUM [80, 2] fp32: A_g[i], B_g[i]. Per
  125-frame chunk the interpolation blend y = w0[i]*A + w1[i]*B runs
  as 2 wide DVE ops (per-partition ramp scalars, stride-2 PSUM APs),
  then a PE-transpose [80,125]->[125,80] puts y in frame-major order,
  ACT evacuates PSUM->SBUF fp32, and one contiguous DMA stores the
  chunk. ldweights dominates PE time (~67ns/frame: stationary load
  cost scales with columns). The 50x shifted-copy replication is ~9MB
  of DMA traffic, so each chunk's load is split across two DGE queues
  (SP half + GpSimd half) to spread DMA-engine load; 6 chunk buffers
  deep so the PE never starves.

Path 2 (DVE+ACT machinery), frames seq0[125:1000):
  fp16 "C-decomposition": C_k[i'] = sum_j h[k,j] x[(k-1)P+i'-j],
  i' in [0,160); y[kP+i] = w0[i]*C_k[80+i] + w1[i]*C_{k+1}[i].
  Per 126-row tile, N_DVE taps run as scalar_tensor_tensor chains on
  Vector; the other taps are Scalar-engine products folded by a fp16
  tensor-tensor halving tree on Vector. Cross-partition combine via
  partition-shifted SBUF->SBUF DMA + one add emitting fp32.

The paths share engines: DVE runs one PE blend after each path-2
tile; ACT runs one PSUM evacuation per tile; SP and GpSimd split the
DMA issue load so neither path's waits block the other's transfers.

Sync design note: cumulative thresholds on a shared DMA semaphore are
unsound with >1 DMA in flight (per-SDMA-engine completion skew lets a
later tile's increments satisfy an earlier tile's threshold). Buffer-
parity semaphores make every threshold equal to the maximum possible
increment count at wait time, so a fired wait implies full completion.
"""

import sys

for p in ("/opt/trn_rl_repo", "/root/.axon_site/_ro/trn_rl_repo"):
    if p not in sys.path:
        sys.path.append(p)

import numpy as np
import concourse.bass as bass
import concourse.mybir as mybir
from concourse.ap import AP
from concourse.bass_utils import run_bass_kernel_spmd

B, T = 16, 80000
P, D = 80, 50  # frame period, taps
N = T // P  # 1000 frames
W2 = 2 * P + D - 1  # 209: extended window for the 160-wide C rows
NCORES = 8
S = B // NCORES  # sequences per core
FO = 125  # output frames per tile (path 2) / per PE chunk
FT = FO + 1  # C-rows per tile (tiles overlap by 1 row)
PAD = D - 1 + P  # front pad so windows are in-bounds: 129
TPC = N * P + W2 + 2  # padded x length (+2 slack for the odd-offset copy)

F16 = mybir.dt.float16
FP32 = mybir.dt.float32

N_DVE = 26  # path-2 taps computed on the Vector engine

# --- PE path layout ---
PE_CHUNKS = [(0, 0)] + [(1, g0) for g0 in range(0, N, FO)]  # (seq, first frame)
NCH = len(PE_CHUNKS)  # 9 chunks x 125 frames
PE_S0_FRAMES = 125  # seq0 frames handled by the PE path
NT_BASE = (N - PE_S0_FRAMES) // FO  # 7 path-2 tiles, all seq0
HTS = 1008  # ht column stride per sequence
# SDMA engine k serves partitions 8k..8k+7, so a 50-partition DMA only uses
# engines 0-6. Each chunk is split into two partition blocks — taps at
# partitions 0:50 (frames 0..GA-1) and 64:114 (frames GA..124) — engaging 14
# engines. Block B matmuls use stationary base partition 64 (legal tile
# position for a <=64-row stationary); ht rows are duplicated there.
GA = 63  # frames served by block A
WXA = GA * P  # block A width: 5040
WXB = (FO - GA) * P  # block B width: 4960
NXS = 8  # chunk buffers (DMA runway depth)

_nc_cache = {}


def build_nc():
    if "nc" in _nc_cache:
        return _nc_cache["nc"]
    nc = bass.Bass()
    xp_ext = nc.declare_dram_parameter("xp", [S, TPC], F16, isOutput=False)
    hc_ext = nc.declare_dram_parameter("hc", [S, N + 1, D], FP32, isOutput=False)
    rr_ext = nc.declare_dram_parameter("rr", [128, 2 * P], F16, isOutput=False)
    ht_ext = nc.declare_dram_parameter("ht", [64 + D, S * HTS], F16, isOutput=False)
    id_ext = nc.declare_dram_parameter("idt", [128, 128], F16, isOutput=False)
    wv_ext = nc.declare_dram_parameter("wv", [128, 2], FP32, isOutput=False)
    out_ext = nc.declare_dram_parameter("out", [S, T], FP32, isOutput=True)

    from contextlib import ExitStack

    with ExitStack() as _ctx:
        ec = _ctx.enter_context
        # --- path 2 (DVE+ACT) buffers ---
        xa0 = ec(nc.sbuf_tensor([FT, W2], F16))
        xa1 = ec(nc.sbuf_tensor([FT, W2], F16))
        xb0 = ec(nc.sbuf_tensor([FT, W2], F16))
        xb1 = ec(nc.sbuf_tensor([FT, W2], F16))
        hh0 = ec(nc.sbuf_tensor([FT, D], FP32))
        hh1 = ec(nc.sbuf_tensor([FT, D], FP32))
        acc0 = ec(nc.sbuf_tensor([FT, 2 * P], F16))
        acc1 = ec(nc.sbuf_tensor([FT, 2 * P], F16))
        vt = ec(nc.sbuf_tensor([FT, 2 * P], F16))
        vs = ec(nc.sbuf_tensor([FO, P], F16))
        y0 = ec(nc.sbuf_tensor([FO, P], FP32))
        y1 = ec(nc.sbuf_tensor([FO, P], FP32))
        rrt = ec(nc.sbuf_tensor([128, 2 * P], F16))
        ramp_sem = ec(nc.semaphore("ramp_sem"))
        dma_e = ec(nc.semaphore("dma_e"))
        dma_o = ec(nc.semaphore("dma_o"))
        v_sem = ec(nc.semaphore("v_sem"))
        vs_sem = ec(nc.semaphore("vs_sem"))
        ya_sem = ec(nc.semaphore("ya_sem"))
        out_e = ec(nc.semaphore("out_e"))
        out_o = ec(nc.semaphore("out_o"))
        act_sem = ec(nc.semaphore("act_sem"))
        N_ACT = D - N_DVE
        NSLOT = 32  # padded to a power of two for the in-place halving tree
        assert N_ACT <= NSLOT
        prb0 = ec(nc.sbuf_tensor([FT, NSLOT * 2 * P], F16))
        prb1 = ec(nc.sbuf_tensor([FT, NSLOT * 2 * P], F16))
        prb = [prb0, prb1]

        # --- PE path buffers ---
        xs = [ec(nc.sbuf_tensor(f"xs{i}", [64 + D, WXA], F16)) for i in range(NXS)]
        htt = ec(nc.sbuf_tensor([64 + D, S * HTS], F16))
        idt = ec(nc.sbuf_tensor([128, 128], F16))
        wvt = ec(nc.sbuf_tensor([128, 2], FP32))
        t1b = ec(nc.sbuf_tensor([P, 128], F16))
        yph = [ec(nc.sbuf_tensor(f"yph{i}", [P, 128], F16)) for i in range(2)]
        yo = [ec(nc.sbuf_tensor(f"yo{i}", [FO, P], FP32)) for i in range(2)]
        pab = [ec(nc.psum_tensor(f"pab{i}", [P, 2 * FO], FP32)) for i in range(4)]
        pT = [ec(nc.psum_tensor(f"pT{i}", [FO, P], F16)) for i in range(2)]
        hts = ec(nc.semaphore("hts"))
        wvs = ec(nc.semaphore("wvs"))
        ids = ec(nc.semaphore("ids"))
        xsd = [ec(nc.semaphore(f"xsd{i}")) for i in range(NXS)]
        pe_mm = ec(nc.semaphore("pe_mm"))
        pe_tr = ec(nc.semaphore("pe_tr"))
        bl_sem = ec(nc.semaphore("bl_sem"))
        ev_sem = ec(nc.semaphore("ev_sem"))
        yst = [ec(nc.semaphore(f"yst{i}")) for i in range(2)]

        block = ec(nc.Block())
        xa = [xa0, xa1]
        xb = [xb0, xb1]
        hh = [hh0, hh1]
        yt = [y0, y1]
        dma_s = [dma_e, dma_o]
        out_s = [out_e, out_o]

        def ci_of(t):
            return t + 1  # path-2 tile t covers seq0 frames [(t+1)*FO, (t+2)*FO)

        def ydst(t):
            ci = ci_of(t)
            return AP(
                tensor=out_ext[:].tensor,
                offset=0 * T + ci * FO * P,
                ap=[[P, FO], [1, P]],
            )

        def xs_dma(eng, c):
            # two partition blocks per chunk; partition p (resp. 64+p) holds
            # x shifted by tap j = D-1-p (ht rows are flipped to match)
            s, g0 = PE_CHUNKS[c]
            src_a = AP(
                tensor=xp_ext[:].tensor,
                offset=s * TPC + PAD + g0 * P - (D - 1),
                ap=[[1, D], [1, WXA]],
            )
            eng.dma_start(xs[c % NXS][0:D, 0:WXA], src_a).then_inc(xsd[c % NXS], 16)
            src_b = AP(
                tensor=xp_ext[:].tensor,
                offset=s * TPC + PAD + (g0 + GA) * P - (D - 1),
                ap=[[1, D], [1, WXB]],
            )
            eng.dma_start(xs[c % NXS][64 : 64 + D, 0:WXB], src_b).then_inc(
                xsd[c % NXS], 16
            )

        @block.sync
        def _(sync):
            # path-2 tile 0/1 inputs go FIRST: SDMA rings drain FIFO per
            # engine, so anything issued after the ~9MB Xs prefetch would
            # stall path-2's start by tens of us.
            def tile_in(t):
                ci = ci_of(t)
                b = t % 2
                k0 = ci * FO
                src_a = AP(
                    tensor=xp_ext[:].tensor,
                    offset=0 * TPC + k0 * P,
                    ap=[[P, FT], [1, W2]],
                )
                src_b = AP(
                    tensor=xp_ext[:].tensor,
                    offset=0 * TPC + k0 * P + 1,
                    ap=[[P, FT], [1, W2]],
                )
                sync.dma_start(xa[b][:], src_a).then_inc(dma_s[b], 16)
                sync.dma_start(xb[b][:], src_b).then_inc(dma_s[b], 16)
                sync.dma_start(hh[b][:], hc_ext[0, k0 : k0 + FT, :]).then_inc(
                    dma_s[b], 16
                )

            tile_in(0)
            sync.dma_start(rrt[:], rr_ext[:]).then_inc(ramp_sem, 16)
            tile_in(1)
            sync.dma_start(htt[:], ht_ext[:]).then_inc(hts, 16)
            xs_dma(sync, 0)
            sync.dma_start(wvt[:], wv_ext[:]).then_inc(wvs, 16)
            sync.dma_start(idt[:], id_ext[:]).then_inc(ids, 16)
            for c in range(1, NXS):
                xs_dma(sync, c)

            def y_store(c):
                s, g0 = PE_CHUNKS[c]
                dst = AP(
                    tensor=out_ext[:].tensor,
                    offset=s * T + g0 * P,
                    ap=[[P, FO], [1, P]],
                )
                sync.dma_start(dst, yo[c % 2][0:FO, 0:P]).then_inc(yst[c % 2], 16)

            for t in range(NT_BASE):
                if t >= 2:
                    sync.wait_ge(v_sem, t - 1)  # WAR: tile t-2 read its inputs
                    tile_in(t)
                if t >= 1:
                    # partition-shift copy of V rows 1..FT for tile t-1
                    sync.wait_ge(v_sem, t)
                    sync.dma_start(vs[:], vt[1:FT, 0:P]).then_inc(vs_sem, 16)
                if t >= 2:
                    # store y of tile t-2
                    sync.wait_ge(ya_sem, t - 1)
                    sync.dma_start(ydst(t - 2), yt[(t - 2) % 2][:]).then_inc(
                        out_s[(t - 2) % 2], 16
                    )
                # --- PE path interleaves ---
                if 1 <= t <= NCH - NXS:
                    c = t + NXS - 1  # remaining Xs chunks
                    sync.wait_ge(pe_mm, c - (NXS - 1))  # buffer c%NXS free
                    xs_dma(sync, c)
                if t >= 2:
                    c = t - 2  # stores for chunks 0..4
                    sync.wait_ge(ev_sem, c + 1)
                    y_store(c)

            # tail: last tile's shift + remaining stores
            tl = NT_BASE - 1
            sync.wait_ge(v_sem, NT_BASE)
            sync.dma_start(vs[:], vt[1:FT, 0:P]).then_inc(vs_sem, 16)
            sync.wait_ge(ya_sem, NT_BASE - 1)
            sync.dma_start(ydst(tl - 1), yt[(tl - 1) % 2][:]).then_inc(
                out_s[(tl - 1) % 2], 16
            )
            sync.wait_ge(ya_sem, NT_BASE)
            sync.dma_start(ydst(tl), yt[tl % 2][:]).then_inc(out_s[tl % 2], 16)
            for c in range(NT_BASE - 2, NCH):
                sync.wait_ge(ev_sem, c + 1)
                y_store(c)
            sync.wait_ge(out_s[tl % 2], 16 * (tl // 2 + 1))
            sync.wait_ge(out_s[1 - tl % 2], 16 * ((tl - 1) // 2 + 1))
            sync.wait_ge(yst[0], 16 * ((NCH + 1) // 2))
            sync.wait_ge(yst[1], 16 * (NCH // 2))

        def src_for(buf_pair, b, j):
            # slice of the extended window for tap j, 4B-aligned via the
            # one-element-shifted copy when the natural offset is odd
            off = D - 1 - j
            if off % 2 == 0:
                return buf_pair[0][b][:, off : off + 2 * P]
            return buf_pair[1][b][:, off - 1 : off - 1 + 2 * P]

        @block.vector
        def _(vector):
            def conv(t):
                b = t % 2
                accs = [acc0, acc1]
                vector.wait_ge(dma_s[b], 48 * (t // 2 + 1))
                vector.tensor_scalar_mul(acc0[:], src_for((xa, xb), b, 0), hh[b][:, 0:1])
                cur = 0
                for j in range(1, N_DVE):
                    nxt = 1 - cur
                    vector.scalar_tensor_tensor(
                        out=accs[nxt][:],
                        in0=src_for((xa, xb), b, j),
                        scalar=hh[b][:, j : j + 1],
                        in1=accs[cur][:],
                        op0=mybir.AluOpType.mult,
                        op1=mybir.AluOpType.add,
                    )
                    cur = nxt
                # fold in the ACT-engine products
                vector.wait_ge(act_sem, t + 1)
                if N_ACT > 16:
                    extra = N_ACT - 16
                    vector.tensor_tensor(
                        out=prb[b][:, 0 : extra * 2 * P],
                        in0=prb[b][:, 0 : extra * 2 * P],
                        in1=prb[b][:, 16 * 2 * P : N_ACT * 2 * P],
                        op=mybir.AluOpType.add,
                    )
                    width = 16 * 2 * P
                else:
                    width = NSLOT * 2 * P
                while width > 2 * P:
                    half = width // 2
                    vector.tensor_tensor(
                        out=prb[b][:, 0:half],
                        in0=prb[b][:, 0:half],
                        in1=prb[b][:, half:width],
                        op=mybir.AluOpType.add,
                    )
                    width = half
                nxt = 1 - cur
                vector.tensor_tensor(
                    out=accs[nxt][:],
                    in0=accs[cur][:],
                    in1=prb[b][:, 0 : 2 * P],
                    op=mybir.AluOpType.add,
                )
                cur = nxt
                return accs[cur]

            def blend(c):
                buf = c % 4
                vector.wait_ge(pe_mm, c + 1)
                if c >= 2:
                    vector.wait_ge(pe_tr, c - 1)  # yph[c%2] WAR
                if c == 0:
                    vector.wait_ge(wvs, 16)
                vector.tensor_scalar_mul(
                    t1b[0:P, 0:FO], pab[buf][0:P, 1 : 2 * FO : 2], wvt[0:P, 1:2]
                )
                vector.scalar_tensor_tensor(
                    out=yph[c % 2][0:P, 0:FO],
                    in0=pab[buf][0:P, 0 : 2 * FO : 2],
                    scalar=wvt[0:P, 0:1],
                    in1=t1b[0:P, 0:FO],
                    op0=mybir.AluOpType.mult,
                    op1=mybir.AluOpType.add,
                ).then_inc(bl_sem, 1)

            if N_ACT <= 16:
                for pp in range(2):
                    vector.memset(prb[pp][:, N_ACT * 2 * P : NSLOT * 2 * P], 0.0)
            for t in range(NT_BASE):
                fin = conv(t)
                if t == 0:
                    vector.wait_ge(ramp_sem, 16)
                if t >= 1:
                    # combine tile t-1: y = V[0:FO, 80:160] + Vs
                    vector.wait_ge(vs_sem, 16 * t)
                    if t - 1 >= 2:
                        vector.wait_ge(out_s[(t - 1) % 2], 16 * ((t - 1) // 2))
                    vector.tensor_tensor(
                        out=yt[(t - 1) % 2][:],
                        in0=vt[0:FO, P : 2 * P],
                        in1=vs[:],
                        op=mybir.AluOpType.add,
                    ).then_inc(ya_sem, 1)
                # V_t = C_t * rr
                vector.tensor_tensor(
                    out=vt[:], in0=fin[:], in1=rrt[0:FT, :], op=mybir.AluOpType.mult
                ).then_inc(v_sem, 1)
                # --- PE-path blend interleave: chunk t after tile t ---
                if t < NCH:
                    blend(t)
            # tail combine for last tile
            tl = NT_BASE - 1
            vector.wait_ge(vs_sem, 16 * NT_BASE)
            vector.wait_ge(out_s[tl % 2], 16 * (tl // 2))
            vector.tensor_tensor(
                out=yt[tl % 2][:],
                in0=vt[0:FO, P : 2 * P],
                in1=vs[:],
                op=mybir.AluOpType.add,
            ).then_inc(ya_sem, 1)
            for c in range(NT_BASE, NCH):
                blend(c)

        @block.scalar
        def _(scalar):
            def evac(c):
                scalar.wait_ge(pe_tr, c + 1)
                if c >= 2:
                    scalar.wait_ge(yst[c % 2], 16 * ((c - 2) // 2 + 1))  # yo WAR
                scalar.activation(
                    yo[c % 2][0:FO, 0:P],
                    pT[c % 2][0:FO, 0:P],
                    mybir.ActivationFunctionType.Copy,
                ).then_inc(ev_sem, 1)

            for t in range(NT_BASE):
                b = t % 2
                scalar.wait_ge(dma_s[b], 48 * (t // 2 + 1))
                if t >= 2:
                    scalar.wait_ge(v_sem, t - 1)  # WAR on prb[b] scratch
                for idx, j in enumerate(range(N_DVE, D)):
                    inst = scalar.activation(
                        prb[b][:, idx * 2 * P : (idx + 1) * 2 * P],
                        src_for((xa, xb), b, j),
                        mybir.ActivationFunctionType.Copy,
                        scale=hh[b][:, j : j + 1],
                    )
                    if idx == N_ACT - 1:
                        inst.then_inc(act_sem, 1)
                # --- PE-path evacuation interleave ---
                c = t - 1
                if 0 <= c < NCH:
                    evac(c)
            for c in range(NT_BASE - 1, NCH):
                evac(c)

        @block.tensor
        def _(tensor):
            def do_transpose(c):
                tensor.wait_ge(bl_sem, c + 1)  # yph ready
                if c == 0:
                    tensor.wait_ge(ids, 16)
                if c >= 2:
                    tensor.wait_ge(ev_sem, c - 1)  # pT[c%2] WAR
                tensor.transpose(
                    pT[c % 2][0:FO, 0:P],
                    yph[c % 2][0:P, 0:FO],
                    idt[0:P, 0:P],
                ).then_inc(pe_tr, 1)

            tensor.wait_ge(hts, 16)
            for c in range(NCH):
                s, g0 = PE_CHUNKS[c]
                buf = c % NXS
                if c >= 4:
                    tensor.wait_ge(bl_sem, c - 3)  # pab[c%4] WAR
                if c >= 3:
                    # transposes trail the mm stream by 3 chunks so the PE
                    # never blocks on a DVE blend mid-stream
                    do_transpose(c - 3)
                tensor.wait_ge(xsd[buf], 32 * (c // NXS + 1))
                for g in range(FO):
                    if g < GA:
                        lhs = xs[buf][0:D, P * g : P * g + P]
                        rhs = htt[0:D, s * HTS + g0 + g : s * HTS + g0 + g + 2]
                    else:
                        lhs = xs[buf][64 : 64 + D, P * (g - GA) : P * (g - GA) + P]
                        rhs = htt[
                            64 : 64 + D, s * HTS + g0 + g : s * HTS + g0 + g + 2
                        ]
                    mm = tensor.matmul(
                        pab[c % 4][0:P, 2 * g : 2 * g + 2],
                        lhs,
                        rhs,
                        start=True,
                        stop=True,
                    )
                    if g == FO - 1:
                        mm.then_inc(pe_mm, 1)
            for c in range(NCH - 3, NCH):
                do_transpose(c)

    _nc_cache["nc"] = nc
    return nc


def _prep_core_inputs(x, h):
    x = np.ascontiguousarray(x, dtype=np.float32)
    h = np.ascontiguousarray(h, dtype=np.float32)
    xp = np.zeros((B, TPC), np.float16)
    xp[:, PAD : PAD + T] = x.astype(np.float16)
    hpad = np.ascontiguousarray(np.concatenate([h, h[:, -1:, :]], axis=1))  # (B,N+1,D) f32
    w1 = (np.arange(P, dtype=np.float32) / P).astype(np.float16)
    w0 = (1.0 - np.arange(P, dtype=np.float32) / P).astype(np.float16)
    rr = np.broadcast_to(np.concatenate([w1, w0])[None, :], (128, 2 * P))
    rr = np.ascontiguousarray(rr)
    hpad16 = hpad.astype(np.float16)  # (B, N+1, D)
    idt = np.eye(128, dtype=np.float16)
    wv = np.zeros((128, 2), np.float32)
    wv[0:P, 0] = 1.0 - np.arange(P, dtype=np.float32) / P
    wv[0:P, 1] = np.arange(P, dtype=np.float32) / P
    in_maps = []
    for c in range(NCORES):
        sl = slice(c * S, (c + 1) * S)
        ht = np.zeros((64 + D, S * HTS), np.float16)
        for s in range(S):
            # row p = tap D-1-p, matching the stride +1 shifted-x layout;
            # duplicated at partition base 64 for block-B matmuls
            ht[0:D, s * HTS : s * HTS + N + 1] = hpad16[c * S + s].T[::-1, :]
            ht[64 : 64 + D, s * HTS : s * HTS + N + 1] = hpad16[c * S + s].T[::-1, :]
        in_maps.append(
            {
                "xp": xp[sl],
                "hc": hpad[sl],
                "rr": rr,
                "ht": ht,
                "idt": idt,
                "wv": wv,
            }
        )
    return in_maps


def kernel(x, h, **kw):
    nc = build_nc()
    in_maps = _prep_core_inputs(x, h)
    res = run_bass_kernel_spmd(nc, in_maps, core_ids=list(range(NCORES)), **kw)
    out = np.concatenate([res.results[c]["out"] for c in range(NCORES)], axis=0)
    return np.ascontiguousarray(out, dtype=np.float32)


def kernel_traced(x, h, **kw):
    nc = build_nc()
    in_maps = _prep_core_inputs(x, h)
    res = run_bass_kernel_spmd(
        nc, in_maps, core_ids=list(range(NCORES)), trace=True, **kw
    )
    out = np.concatenate([res.results[c]["out"] for c in range(NCORES)], axis=0)
    return np.ascontiguousarray(out, dtype=np.float32), res
